# revision 7
# baseline (speedup 1.0000x reference)
"""CapsLayer dynamic-routing kernel for 8x Trainium2 (Bass/Tile).

Strategy:
  - Shard the 16 capsules across 8 cores (2 per core); batch stays whole.
    The routing loop is independent per (capsule, batch) so there is no
    cross-core communication; route_weights are the dominant traffic and
    this sharding reads them exactly once per core (37.7MB vs 302MB for
    batch sharding).
  - Per capsule, priors P[b,n,o] are materialized in SBUF in fp16+fp16
    residual pairs (numerically ~fp32: validated 1.8e-7 matmul error),
    laid out as [part = o + 64*(n%2), free = b*576 + n//2].
  - All contractions run on the PE at 1 cyc/row fp16 via 3-term splits
    (hi*hi + hi*lo + lo*hi).  Softmax uses a constant shift (softmax is
    shift invariant; max logit ~130 < 60+88) so exp needs no max pass.
  - Per-iteration logits are computed as P @ cumulative-v (cheaper than
    storing logits), broadcast across partitions by a "fat" block-diagonal
    lhsT so the exp output is already replicated for the weighted-sum
    (S = sum_n softmax_n * P) which runs as a fused multiply+row-sum
    (scalar_tensor_tensor accum_out) on the vector engine.
"""

import os
import sys

import numpy as np

for _p in ("/opt/trn_rl_repo", "/root/.axon_site/_ro/trn_rl_repo"):
    if os.path.isdir(_p) and _p not in sys.path:
        sys.path.insert(0, _p)

import concourse.bass as bass  # noqa: E402
import concourse.mybir as mybir  # noqa: E402
from concourse import bacc  # noqa: E402
from concourse.bass_utils import run_bass_kernel_spmd  # noqa: E402
from concourse.masks import make_identity  # noqa: E402
from concourse.tile import TileContext  # noqa: E402

C, B, N, I, O = 16, 64, 1152, 64, 64
NCORES = 8
CPC = C // NCORES          # capsules per core
N2 = N // 2                # 576 n-pairs (parity-interleaved partitions)
CH = 8                     # n-pairs per creation chunk
NCH = N2 // CH             # 72 chunks
NJ = 2 * CH                # 16 route nodes per chunk
SHIFT = 60.0               # constant softmax shift (range guard only)
F16 = mybir.dt.float16
F32 = mybir.dt.float32
ADD = mybir.AluOpType.add
MUL = mybir.AluOpType.mult
SUB = mybir.AluOpType.subtract

TRACE = [False]            # test.py flips this for the profiling run
LAST_RESULT = {}


def build_program():
    nc = bacc.Bacc("TRN2", target_bir_lowering=False, debug=False,
                   num_devices=NCORES)
    wp = nc.declare_dram_parameter("wp", [CPC, NCH, I, NJ, 128], F16,
                                   isOutput=False)
    xp = nc.declare_dram_parameter("xp", [NCH, I, NJ, 128], F16,
                                   isOutput=False)
    out_d = nc.declare_dram_parameter("out", [CPC, B, O], F32, isOutput=True)

    with TileContext(nc) as tc:
        with (
            tc.tile_pool(name="pbig", bufs=1) as pbig,
            tc.tile_pool(name="pconst", bufs=1) as pconst,
            tc.tile_pool(name="pw", bufs=3) as pw,
            tc.tile_pool(name="px", bufs=3) as px,
            tc.tile_pool(name="pcr", bufs=2, space="PSUM") as pcr,
            tc.tile_pool(name="pd", bufs=1, space="PSUM") as pd,
            tc.tile_pool(name="pst", bufs=1, space="PSUM") as pst,
            tc.tile_pool(name="pe", bufs=3) as pe,
            tc.tile_pool(name="pfat", bufs=4) as pfat,
            tc.tile_pool(name="ptr", bufs=2) as ptr,
            tc.tile_pool(name="psm", bufs=2) as psm,
        ):
            id64 = pconst.tile([64, 64], F32, tag="id64")
            make_identity(nc, id64[:, :])
            stack2 = pconst.tile([128, 64], F32, tag="stack2")
            nc.gpsimd.memset(stack2[:, :], 0.0)
            make_identity(nc, stack2[0:64, :], nomemset=True)
            make_identity(nc, stack2[64:128, :], nomemset=True)
            bigmask = pconst.tile([128, 128], F16, tag="bigmask")
            nc.gpsimd.memset(bigmask[:, :], 0.0)
            nc.gpsimd.memset(bigmask[0:64, 0:64], 1.0)
            nc.gpsimd.memset(bigmask[64:128, 64:128], 1.0)
            shiftb = pconst.tile([128, 1], F32, tag="shiftb")
            nc.vector.memset(shiftb[:, :], -SHIFT)

            for cc in range(CPC):
                p16 = pbig.tile([128, 64 * N2], F16, tag="p16")
                pr = pbig.tile([128, 64 * N2], F16, tag="pr")
                p16v = p16[:, :].rearrange("p (b n) -> p b n", b=64)
                prv = pr[:, :].rearrange("p (b n) -> p b n", b=64)

                # ---------------- creation: P = x @ W per route node -------
                for ci in range(NCH):
                    wt = pw.tile([I, NJ, 128], F16, tag="wt")
                    nc.sync.dma_start(out=wt[:, :, :], in_=wp[cc, ci])
                    xt = px.tile([I, NJ, 128], F16, tag="xt")
                    nc.sync.dma_start(out=xt[:, :, :], in_=xp[ci])
                    ps = pcr.tile([128, CH, 64], F32, tag="cps")
                    for j in range(NJ):
                        jj, par = j // 2, j % 2
                        oap = ps[64 * par:64 * par + 64, jj, :]
                        w16 = wt[:, j, 0:64]
                        wr = wt[:, j, 64:128]
                        x16 = xt[:, j, 0:64]
                        xr = xt[:, j, 64:128]
                        nc.tensor.matmul(oap, lhsT=w16, rhs=x16,
                                         start=True, stop=False)
                        nc.tensor.matmul(oap, lhsT=wr, rhs=x16,
                                         start=False, stop=False)
                        nc.tensor.matmul(oap, lhsT=w16, rhs=xr,
                                         start=False, stop=True)
                    psv = ps[:, :, :].rearrange("p j b -> p b j")
                    d16 = p16v[:, :, ci * CH:(ci + 1) * CH]
                    dr = prv[:, :, ci * CH:(ci + 1) * CH]
                    nc.scalar.copy(d16, psv)
                    nc.vector.tensor_tensor(dr, psv, d16, SUB)

                # ---------------- U = sum_n P (uniform-probs first iter) ---
                u = psm.tile([128, 64], F32, tag="u")
                ur = psm.tile([128, 64], F32, tag="ur")
                nc.vector.tensor_reduce(u[:, :], p16v[:, :, :],
                                        axis=mybir.AxisListType.X, op=ADD)
                nc.vector.tensor_reduce(ur[:, :], prv[:, :, :],
                                        axis=mybir.AxisListType.X, op=ADD)
                nc.vector.tensor_tensor(u[:, :], u[:, :], ur[:, :], ADD)

                vc = psm.tile([64, 64], F32, tag="vcum")
                nc.vector.memset(vc[:, :], 0.0)

                def boundary(scol, zsum, final, vc=vc, cc=cc):
                    """Parity-combine + squash.  scol/zsum: [128,64] f32."""
                    cps = pst.tile([64, 128], F32, tag="cps2")
                    nc.tensor.matmul(cps[:, 0:64], lhsT=stack2[:, :],
                                     rhs=scol[:, :], start=True, stop=True)
                    if zsum is not None:
                        nc.tensor.matmul(cps[:, 64:128], lhsT=stack2[:, :],
                                         rhs=zsum[:, :], start=True, stop=True)
                    sb1 = psm.tile([64, 128], F32, tag="sb1")
                    nc.scalar.copy(sb1[:, :], cps[:, :])
                    t1 = pst.tile([64, 64], F32, tag="t1")
                    nc.tensor.transpose(t1[:, :], sb1[:, 0:64], id64[:, :])
                    sbt = psm.tile([64, 128], F32, tag="sbt")
                    nc.scalar.copy(sbt[:, 0:64], t1[:, :])
                    zb = psm.tile([64, 1], F32, tag="zb")
                    if zsum is not None:
                        t2 = pst.tile([64, 64], F32, tag="t2")
                        nc.tensor.transpose(t2[:, :], sb1[:, 64:128],
                                            id64[:, :])
                        nc.scalar.copy(zb[:, :], t2[:, 0:1])
                    else:
                        nc.vector.memset(zb[:, :], float(N))
                    zinv = psm.tile([64, 1], F32, tag="zinv")
                    nc.vector.reciprocal(zinv[:, :], zb[:, :])
                    ss = psm.tile([64, 64], F32, tag="ss")
                    nc.vector.tensor_scalar_mul(ss[:, :], sbt[:, 0:64],
                                                zinv[:, :])
                    nsq = psm.tile([64, 1], F32, tag="nsq")
                    tr64 = psm.tile([64, 64], F32, tag="tr64")
                    nc.vector.scalar_tensor_tensor(
                        out=tr64[:, :], in0=ss[:, :], scalar=1.0,
                        in1=ss[:, :], op0=MUL, op1=MUL,
                        accum_out=nsq[:, :])
                    den = psm.tile([64, 1], F32, tag="den")
                    nc.vector.tensor_scalar_add(den[:, :], nsq[:, :], 1.0)
                    inv = psm.tile([64, 1], F32, tag="inv")
                    nc.vector.reciprocal(inv[:, :], den[:, :])
                    sq = psm.tile([64, 1], F32, tag="sq")
                    nc.scalar.activation(sq[:, :], nsq[:, :],
                                         mybir.ActivationFunctionType.Sqrt)
                    coef = psm.tile([64, 1], F32, tag="coef")
                    nc.vector.tensor_tensor(coef[:, :], sq[:, :], inv[:, :],
                                            MUL)
                    vbo = psm.tile([64, 64], F32, tag="vbo")
                    nc.vector.tensor_scalar_mul(vbo[:, :], ss[:, :],
                                                coef[:, :])
                    if final:
                        nc.sync.dma_start(out=out_d[cc], in_=vbo[:, :])
                        return None
                    nc.vector.tensor_tensor(vc[:, :], vc[:, :], vbo[:, :],
                                            ADD)
                    vc2 = psm.tile([64, 128], F32, tag="vc2")
                    nc.vector.tensor_copy(vc2[:, 0:64], vc[:, :])
                    nc.vector.tensor_copy(vc2[:, 64:128], vc[:, :])
                    tv = pst.tile([128, 64], F32, tag="tv")
                    nc.tensor.transpose(tv[:, :], vc2[:, :], id64[:, :])
                    v16h = psm.tile([128, 64], F16, tag="v16h")
                    nc.scalar.copy(v16h[:, :], tv[:, :])
                    v216 = psm.tile([128, 64], F32, tag="v216")
                    nc.scalar.copy(v216[:, :], v16h[:, :])
                    v2r = psm.tile([128, 64], F32, tag="v2r")
                    nc.vector.tensor_tensor(v2r[:, :], tv[:, :], v216[:, :],
                                            SUB)
                    return v216, v2r

                def routing_pass(v216, v2r):
                    """logits = P @ vcum (broadcast), E = exp(l-60),
                    S~ = sum_n E*P16, Z = sum_n E.  Returns [128,64] cols."""
                    scol = psm.tile([128, 64], F32, tag="scol")
                    zca = psm.tile([128, 64], F32, tag="zca")
                    zcb = psm.tile([128, 64], F32, tag="zcb")
                    for b in range(B):
                        fv16 = pfat.tile([128, 128], F16, tag="fv16")
                        nc.vector.tensor_scalar_mul(fv16[:, :],
                                                    bigmask[:, :],
                                                    v216[:, b:b + 1])
                        fvr = pfat.tile([128, 128], F16, tag="fvr")
                        nc.vector.tensor_scalar_mul(fvr[:, :], bigmask[:, :],
                                                    v2r[:, b:b + 1])
                        da = pd.tile([128, 288], F32, tag="da")
                        db = pd.tile([128, 288], F32, tag="db")
                        for dps, sl in ((da, slice(0, 288)),
                                        (db, slice(288, 576))):
                            r16 = p16v[:, b, sl]
                            rr = prv[:, b, sl]
                            nc.tensor.matmul(dps[:, :], lhsT=fv16[:, :],
                                             rhs=r16, start=True, stop=False)
                            nc.tensor.matmul(dps[:, :], lhsT=fvr[:, :],
                                             rhs=r16, start=False, stop=False)
                            nc.tensor.matmul(dps[:, :], lhsT=fv16[:, :],
                                             rhs=rr, start=False, stop=True)
                        eb = pe.tile([128, 576], F32, tag="eb")
                        nc.scalar.activation(
                            eb[:, 0:288], da[:, :],
                            mybir.ActivationFunctionType.Exp,
                            bias=shiftb[:, :], scale=1.0,
                            accum_out=zca[:, b:b + 1])
                        nc.scalar.activation(
                            eb[:, 288:576], db[:, :],
                            mybir.ActivationFunctionType.Exp,
                            bias=shiftb[:, :], scale=1.0,
                            accum_out=zcb[:, b:b + 1])
                        trash = ptr.tile([128, 576], F32, tag="trash")
                        nc.vector.scalar_tensor_tensor(
                            out=trash[:, :], in0=eb[:, :], scalar=1.0,
                            in1=p16v[:, b, :], op0=MUL, op1=MUL,
                            accum_out=scol[:, b:b + 1])
                    zsum = psm.tile([128, 64], F32, tag="zsum")
                    nc.vector.tensor_tensor(zsum[:, :], zca[:, :], zcb[:, :],
                                            ADD)
                    return scol, zsum

                vpair = boundary(u, None, False)
                scol, zsum = routing_pass(*vpair)
                vpair = boundary(scol, zsum, False)
                scol, zsum = routing_pass(*vpair)
                boundary(scol, zsum, True)

    nc.compile()
    return nc


_CACHE = {}


def _get_program():
    if "nc" not in _CACHE:
        _CACHE["nc"] = build_program()
    return _CACHE["nc"]


def _split16(a):
    hi = a.astype(np.float16)
    lo = (a - hi.astype(np.float32)).astype(np.float16)
    return hi, lo


def _pack_nio(hi, lo):
    """[N,I,D] fp16 pair -> [NCH, I, NJ, 128] (node-major chunks)."""
    h = hi.reshape(NCH, NJ, I, 64).transpose(0, 2, 1, 3)
    l = lo.reshape(NCH, NJ, I, 64).transpose(0, 2, 1, 3)
    out = np.empty((NCH, I, NJ, 128), np.float16)
    out[..., 0:64] = h
    out[..., 64:128] = l
    return out


def kernel(x, route_weights):
    x = np.asarray(x, dtype=np.float32)
    rw = np.asarray(route_weights, dtype=np.float32)

    x16, xr = _split16(x)                       # [B,N,I]
    xp = _pack_nio(x16.transpose(1, 2, 0),      # -> [N,I,B]
                   xr.transpose(1, 2, 0))

    in_maps = []
    for core in range(NCORES):
        wpc = np.empty((CPC, NCH, I, NJ, 128), np.float16)
        for ccc in range(CPC):
            c = core * CPC + ccc
            w16, wr = _split16(rw[c])           # [N,I,O]
            wpc[ccc] = _pack_nio(w16, wr)
        in_maps.append({"wp": wpc, "xp": xp})

    nc = _get_program()
    r = run_bass_kernel_spmd(nc, in_maps, list(range(NCORES)),
                             trace=TRACE[0])
    LAST_RESULT["r"] = r
    out = np.empty((C, B, 1, O), np.float32)
    for core in range(NCORES):
        o = r.results[core]["out"]
        for ccc in range(CPC):
            out[core * CPC + ccc, :, 0, :] = o[ccc]
    return out


# revision 28
# speedup vs baseline: 2.8497x; 2.8497x over previous
"""CapsLayer dynamic-routing kernel for 8x Trainium2 (Bass/Tile).

Strategy:
  - Shard the 16 capsules across 8 cores (2 per core); batch stays whole.
    The routing loop is independent per (capsule, batch) so there is no
    cross-core communication; route_weights are the dominant traffic and
    this sharding reads them exactly once per core (37.7MB vs 302MB for
    batch sharding).
  - Per capsule, priors P[b,n,o] are materialized in SBUF in fp16+fp16
    residual pairs (numerically ~fp32: validated 1.8e-7 matmul error),
    laid out as [part = o + 64*(n%2), free = b*576 + n//2].
  - All contractions run on the PE at 1 cyc/row fp16 via 3-term splits
    (hi*hi + hi*lo + lo*hi).  Softmax uses a constant shift (softmax is
    shift invariant; max logit ~130 < 60+88) so exp needs no max pass.
  - Per-iteration logits are computed as P @ cumulative-v (cheaper than
    storing logits), broadcast across partitions by a "fat" block-diagonal
    lhsT so the exp output is already replicated for the weighted-sum
    (S = sum_n softmax_n * P) which runs as a fused multiply+row-sum
    (scalar_tensor_tensor accum_out) on the vector engine.
"""

import os
import sys

import numpy as np

for _p in ("/opt/trn_rl_repo", "/root/.axon_site/_ro/trn_rl_repo"):
    if os.path.isdir(_p) and _p not in sys.path:
        sys.path.insert(0, _p)

import concourse.bass as bass  # noqa: E402
import concourse.mybir as mybir  # noqa: E402
from concourse import bacc  # noqa: E402
from concourse.bass_utils import run_bass_kernel_spmd  # noqa: E402
from concourse.masks import make_identity  # noqa: E402
from concourse.tile import TileContext  # noqa: E402

C, B, N, I, O = 16, 64, 1152, 64, 64
NCORES = 8
CPC = C // NCORES          # capsules per core
N2 = N // 2                # 576 n-pairs (parity-interleaved partitions)
CH = 8                     # n-pairs per creation chunk
NCH = N2 // CH             # 72 chunks
NJ = 2 * CH                # 16 route nodes per chunk
SHIFT = 60.0               # constant softmax shift (range guard only)
F16 = mybir.dt.float16
F32 = mybir.dt.float32
ADD = mybir.AluOpType.add
MUL = mybir.AluOpType.mult
SUB = mybir.AluOpType.subtract

TRACE = [False]            # test.py flips this for the profiling run
LAST_RESULT = {}


def build_program():
    nc = bacc.Bacc("TRN2", target_bir_lowering=False, debug=False,
                   num_devices=NCORES)
    wp = nc.declare_dram_parameter("wp", [CPC, NCH // 2, 128, NJ * 192], F16,
                                   isOutput=False)
    xp = nc.declare_dram_parameter("xp", [NCH // 2, 128, NJ * 128], F16,
                                   isOutput=False)
    out_d = nc.declare_dram_parameter("out", [CPC, B, O], F32, isOutput=True)

    with TileContext(nc) as tc:
        with (
            tc.tile_pool(name="pbig", bufs=1) as pbig,
            tc.tile_pool(name="pconst", bufs=1) as pconst,
            tc.tile_pool(name="pw", bufs=3) as pw,
            tc.tile_pool(name="px", bufs=3) as px,
            tc.tile_pool(name="pcr", bufs=3, space="PSUM") as pcr,
            tc.tile_pool(name="pd", bufs=2, space="PSUM") as pd,
            tc.tile_pool(name="pst", bufs=1, space="PSUM") as pst,
            tc.tile_pool(name="pe", bufs=3) as pe,
            tc.tile_pool(name="pfat", bufs=4) as pfat,
            tc.tile_pool(name="ptr", bufs=3) as ptr,
            tc.tile_pool(name="psm", bufs=2) as psm,
        ):
            id64 = pconst.tile([64, 64], F32, tag="id64")
            make_identity(nc, id64[:, :])
            stack2 = pconst.tile([128, 64], F32, tag="stack2")
            nc.gpsimd.memset(stack2[:, :], 0.0)
            make_identity(nc, stack2[0:64, :], nomemset=True)
            make_identity(nc, stack2[64:128, :], nomemset=True)
            bigmask = pconst.tile([128, 128], F16, tag="bigmask")
            nc.gpsimd.memset(bigmask[:, :], 0.0)
            nc.gpsimd.memset(bigmask[0:64, 0:64], 1.0)
            nc.gpsimd.memset(bigmask[64:128, 64:128], 1.0)
            shiftb = pconst.tile([128, 1], F32, tag="shiftb")
            nc.vector.memset(shiftb[:, :], -SHIFT)
            ones = pconst.tile([128, N2], F16, tag="ones")
            nc.vector.memset(ones[:, :], 1.0)

            for cc in range(CPC):
                p16 = pbig.tile([128, 64 * N2], F16, tag="p16")
                pr = pbig.tile([128, 64 * N2], F16, tag="pr")
                p16v = p16[:, :].rearrange("p (b n) -> p b n", b=64)
                prv = pr[:, :].rearrange("p (b n) -> p b n", b=64)

                # ---------------- creation: P = x @ W per route node -------
                # super-chunks: 128-partition DMAs (full port width); rows
                # 64h..64h+63 hold chunk 2*cj+h's [i] slab.
                # W slab cols per j: [mains(64) | corr-a(64) | corr-b(64)];
                # x slab cols: [a-stack(64) | b-stack(64)].  The correction
                # matmul packs (x16*Wr + xr*W16) into one K=128 pass.
                for cj in range(NCH // 2):
                    wt = pw.tile([128, NJ, 192], F16, tag="wt")
                    nc.gpsimd.dma_start(
                        out=wt[:, :, :],
                        in_=wp[cc, cj].rearrange("p (j c) -> p j c", j=NJ))
                    xt = px.tile([128, NJ, 128], F16, tag="xt")
                    nc.sync.dma_start(
                        out=xt[:, :, :],
                        in_=xp[cj].rearrange("p (j c) -> p j c", j=NJ))
                    for h in range(2):
                        ci = 2 * cj + h
                        hs = 64 * h
                        ps = pcr.tile([128, CH, 64], F32, tag="cps")
                        for j in range(NJ):
                            jj, par = j // 2, j % 2
                            oap = ps[64 * par:64 * par + 64, jj, :]
                            nc.tensor.matmul(
                                oap, lhsT=wt[hs:hs + 64, j, 0:64],
                                rhs=xt[hs:hs + 64, j, hs:hs + 64],
                                start=True, stop=False)
                            nc.tensor.matmul(
                                oap, lhsT=wt[:, j, 64 + hs:128 + hs],
                                rhs=xt[:, j, hs:hs + 64],
                                start=False, stop=True)
                        psv = ps[:, :, :].rearrange("p j b -> p b j")
                        d16 = p16v[:, :, ci * CH:(ci + 1) * CH]
                        dr = prv[:, :, ci * CH:(ci + 1) * CH]
                        nc.scalar.copy(d16, psv)
                        nc.vector.tensor_tensor(dr, psv, d16, SUB)

                # ---------------- U = sum_n P (uniform-probs first iter) ---
                # fused multiply(ones)+rowsum STTs stay in the 16-bit fast
                # mode; PR's half goes to the otherwise-idle GPSIMD.
                u = psm.tile([128, 64], F32, tag="u")
                ur = psm.tile([128, 64], F32, tag="ur")
                for b in range(B):
                    tr1 = ptr.tile([128, N2], F16, tag="trash16")
                    nc.vector.scalar_tensor_tensor(
                        out=tr1[:, :], in0=p16v[:, b, :], scalar=1.0,
                        in1=ones[:, :], op0=MUL, op1=MUL,
                        accum_out=u[:, b:b + 1])
                    tr2 = ptr.tile([128, N2], F16, tag="trash16g")
                    nc.scalar.activation(
                        tr2[:, :], prv[:, b, :],
                        mybir.ActivationFunctionType.Copy,
                        accum_out=ur[:, b:b + 1])
                nc.vector.tensor_tensor(u[:, :], u[:, :], ur[:, :], ADD)

                vc = psm.tile([64, 64], F32, tag="vcum")
                nc.vector.memset(vc[:, :], 0.0)

                def boundary(scol, zsum, final, vc=vc, cc=cc):
                    """Parity-combine + squash.  scol/zsum: [128,64] f32."""
                    cps = pst.tile([64, 128], F32, tag="bnd")
                    nc.tensor.matmul(cps[:, 0:64], lhsT=stack2[:, :],
                                     rhs=scol[:, :], start=True, stop=True)
                    if zsum is not None:
                        nc.tensor.matmul(cps[:, 64:128], lhsT=stack2[:, :],
                                         rhs=zsum[:, :], start=True, stop=True)
                    sb1 = psm.tile([64, 128], F32, tag="sb1")
                    if zsum is not None:
                        nc.scalar.copy(sb1[:, :], cps[:, :])
                    else:
                        nc.scalar.copy(sb1[:, 0:64], cps[:, 0:64])
                    t1 = pst.tile([64, 64], F32, tag="bnd")
                    nc.tensor.transpose(t1[:, :], sb1[:, 0:64], id64[:, :])
                    sbt = psm.tile([64, 128], F32, tag="sbt")
                    nc.scalar.copy(sbt[:, 0:64], t1[:, :])
                    zb = psm.tile([64, 1], F32, tag="zb")
                    if zsum is not None:
                        t2 = pst.tile([64, 64], F32, tag="bnd")
                        nc.tensor.transpose(t2[:, :], sb1[:, 64:128],
                                            id64[:, :])
                        nc.scalar.copy(zb[:, :], t2[:, 0:1])
                    else:
                        nc.vector.memset(zb[:, :], float(N))
                    zinv = psm.tile([64, 1], F32, tag="zinv")
                    nc.vector.reciprocal(zinv[:, :], zb[:, :])
                    ss = psm.tile([64, 64], F32, tag="ss")
                    nc.vector.tensor_scalar_mul(ss[:, :], sbt[:, 0:64],
                                                zinv[:, :])
                    nsq = psm.tile([64, 1], F32, tag="nsq")
                    tr64 = psm.tile([64, 64], F32, tag="tr64")
                    nc.vector.scalar_tensor_tensor(
                        out=tr64[:, :], in0=ss[:, :], scalar=1.0,
                        in1=ss[:, :], op0=MUL, op1=MUL,
                        accum_out=nsq[:, :])
                    den = psm.tile([64, 1], F32, tag="den")
                    nc.vector.tensor_scalar_add(den[:, :], nsq[:, :], 1.0)
                    inv = psm.tile([64, 1], F32, tag="inv")
                    nc.vector.reciprocal(inv[:, :], den[:, :])
                    sq = psm.tile([64, 1], F32, tag="sq")
                    nc.scalar.activation(sq[:, :], nsq[:, :],
                                         mybir.ActivationFunctionType.Sqrt)
                    coef = psm.tile([64, 1], F32, tag="coef")
                    nc.vector.tensor_tensor(coef[:, :], sq[:, :], inv[:, :],
                                            MUL)
                    vbo = psm.tile([64, 64], F32, tag="vbo")
                    nc.vector.tensor_scalar_mul(vbo[:, :], ss[:, :],
                                                coef[:, :])
                    if final:
                        nc.sync.dma_start(out=out_d[cc], in_=vbo[:, :])
                        return None
                    nc.vector.tensor_tensor(vc[:, :], vc[:, :], vbo[:, :],
                                            ADD)
                    vc2 = psm.tile([64, 128], F32, tag="vc2")
                    nc.vector.tensor_copy(vc2[:, 0:64], vc[:, :])
                    nc.vector.tensor_copy(vc2[:, 64:128], vc[:, :])
                    tv = pst.tile([128, 64], F32, tag="bnd")
                    nc.tensor.transpose(tv[:, :], vc2[:, :], id64[:, :])
                    v16h = psm.tile([128, 64], F16, tag="v16h")
                    nc.scalar.copy(v16h[:, :], tv[:, :])
                    v216 = psm.tile([128, 64], F32, tag="v216")
                    nc.scalar.copy(v216[:, :], v16h[:, :])
                    v2r = psm.tile([128, 64], F32, tag="v2r")
                    nc.vector.tensor_tensor(v2r[:, :], tv[:, :], v216[:, :],
                                            SUB)
                    return v216, v2r

                def routing_pass(v216, v2r):
                    """logits = P @ vcum (broadcast), E = exp(l-60),
                    S~ = sum_n E*P16, Z = sum_n E.  Returns [128,64] cols."""
                    scol = psm.tile([128, 64], F32, tag="scol")
                    zc = psm.tile([128, 64], F32, tag="zc")
                    for b in range(B):
                        fv16 = pfat.tile([128, 128], F16, tag="fv16")
                        nc.vector.tensor_scalar_mul(fv16[:, :],
                                                    bigmask[:, :],
                                                    v216[:, b:b + 1])
                        fvr = pfat.tile([128, 128], F16, tag="fvr")
                        nc.vector.tensor_scalar_mul(fvr[:, :], bigmask[:, :],
                                                    v2r[:, b:b + 1])
                        # one [128,1024] tile = 2 psum banks; matmul halves
                        # live at bank-aligned offsets 0 and 512.
                        dd = pd.tile([128, 1024], F32, tag="dd")
                        for off, sl in ((0, slice(0, 288)),
                                        (512, slice(288, 576))):
                            dps = dd[:, off:off + 288]
                            r16 = p16v[:, b, sl]
                            rr = prv[:, b, sl]
                            nc.tensor.matmul(dps, lhsT=fv16[:, :],
                                             rhs=r16, start=True, stop=False)
                            nc.tensor.matmul(dps, lhsT=fvr[:, :],
                                             rhs=r16, start=False, stop=False)
                            nc.tensor.matmul(dps, lhsT=fv16[:, :],
                                             rhs=rr, start=False, stop=True)
                        eb = pe.tile([128, 576], F32, tag="eb")
                        ddv = dd[:, :].rearrange("p (h x) -> p h x",
                                                 h=2)[:, :, 0:288]
                        nc.scalar.activation(
                            eb[:, :].rearrange("p (h x) -> p h x", h=2), ddv,
                            mybir.ActivationFunctionType.Exp,
                            bias=shiftb[:, :], scale=1.0,
                            accum_out=zc[:, b:b + 1])
                        trash = ptr.tile([128, 576], F32, tag="trash")
                        nc.vector.scalar_tensor_tensor(
                            out=trash[:, :], in0=eb[:, :], scalar=1.0,
                            in1=p16v[:, b, :], op0=MUL, op1=MUL,
                            accum_out=scol[:, b:b + 1])
                    return scol, zc

                vpair = boundary(u, None, False)
                scol, zsum = routing_pass(*vpair)
                vpair = boundary(scol, zsum, False)
                scol, zsum = routing_pass(*vpair)
                boundary(scol, zsum, True)

    nc.compile()
    return nc


_CACHE = {}


def _get_program():
    if "nc" not in _CACHE:
        _CACHE["nc"] = build_program()
    return _CACHE["nc"]


def _split16(a):
    hi = a.astype(np.float16)
    lo = (a - hi.astype(np.float32)).astype(np.float16)
    return hi, lo


def _pack_w(hi, lo):
    """[N,I,O] fp16 pair -> [NCH//2, 128, NJ*192] W slabs."""
    h = hi.reshape(NCH, NJ, I, 64)
    l = lo.reshape(NCH, NJ, I, 64)
    ha, hb = h[0::2], h[1::2]
    la, lb = l[0::2], l[1::2]
    out = np.empty((NCH // 2, NJ, 128, 192), np.float16)
    out[:, :, 0:64, 0:64] = ha
    out[:, :, 64:128, 0:64] = hb
    out[:, :, 0:64, 64:128] = la      # corr-a top: Wr_a (pairs x16_a)
    out[:, :, 64:128, 64:128] = ha    # corr-a bot: W16_a (pairs xr_a)
    out[:, :, 0:64, 128:192] = hb     # corr-b top: W16_b (pairs xr_b)
    out[:, :, 64:128, 128:192] = lb   # corr-b bot: Wr_b (pairs x16_b)
    return out.transpose(0, 2, 1, 3).reshape(NCH // 2, 128, NJ * 192)


def _pack_x(hi, lo):
    """[N,I,B] fp16 pair -> [NCH//2, 128, NJ*128] x stacks."""
    h = hi.reshape(NCH, NJ, I, 64)
    l = lo.reshape(NCH, NJ, I, 64)
    out = np.empty((NCH // 2, NJ, 128, 128), np.float16)
    out[:, :, 0:64, 0:64] = h[0::2]       # a-stack top: x16_a
    out[:, :, 64:128, 0:64] = l[0::2]     # a-stack bot: xr_a
    out[:, :, 0:64, 64:128] = l[1::2]     # b-stack top: xr_b
    out[:, :, 64:128, 64:128] = h[1::2]   # b-stack bot: x16_b
    return out.transpose(0, 2, 1, 3).reshape(NCH // 2, 128, NJ * 128)


def kernel(x, route_weights):
    x = np.asarray(x, dtype=np.float32)
    rw = np.asarray(route_weights, dtype=np.float32)

    x16, xr = _split16(x)                       # [B,N,I]
    xp = _pack_x(x16.transpose(1, 2, 0),        # -> [N,I,B]
                 xr.transpose(1, 2, 0))

    in_maps = []
    for core in range(NCORES):
        wpc = np.empty((CPC, NCH // 2, 128, NJ * 192), np.float16)
        for ccc in range(CPC):
            c = core * CPC + ccc
            w16, wr = _split16(rw[c])           # [N,I,O]
            wpc[ccc] = _pack_w(w16, wr)
        in_maps.append({"wp": wpc, "xp": xp})

    nc = _get_program()
    r = run_bass_kernel_spmd(nc, in_maps, list(range(NCORES)),
                             trace=TRACE[0])
    LAST_RESULT["r"] = r
    out = np.empty((C, B, 1, O), np.float32)
    for core in range(NCORES):
        o = r.results[core]["out"]
        for ccc in range(CPC):
            out[core * CPC + ccc, :, 0, :] = o[ccc]
    return out
